# revision 26
# baseline (speedup 1.0000x reference)
"""Trainium2 Bass kernel for CellSegmentationLoss.

For pred_masks x (logits, fp32 [16,1,1024,1024]), gt_masks t (binary fp32),
pred_iou [16,1]:

    ce    = softplus(x) - x*t
    p     = sigmoid(x)
    focal = mean(alpha_t * ce * (1-p_t)^2),  alpha_t = 0.75-0.5t
    dice  = 1 - mean_s (2*sum(p*t)+eps)/(sum(p)+sum(t)+eps)
    boundary = 2*mean(ce)
    iou_loss = mse(pred_iou, actual_iou of thresholded masks)
    loss  = focal + dice + 0.5*boundary + 0.1*iou_loss

Identity (t binary): with zbar = (2t-1)*x
    q  = sigmoid(zbar) = p_t          (ACT pass 1, accum -> sum q)
    l  = ln(q)        = -ce           (ACT pass 2, accum -> -sum ce)
    r  = 1 - q        = sigmoid(-zbar)
    u  = l*r = -ce*r ; gbar = u*r = -ce*r^2 = -(focal modulator g)
    bin = (x > 0) = (p > 0.5)

All needed reductions:
    global: sum q, sum l, per-sample: sum bin (DVE/ACT accum_out, free)
    per-sample sum gbar (PE, ones-stationary streaming gbar)
    t-weighted per-sample sums via PE diag matmuls with STATIONARY
    sbar = 2t-1 (exact +-1): sum sbar*v = 2*sum(t*v) - sum(v), plus a
    1-column ones matmul per chunk giving column sums of sbar -> sum t.

Engine plan per core (2 samples, 16384 free columns):
    ACT: 2 passes (Sigmoid table, then Ln table; one explicit table load
         each -> no thrash)
    DVE: 6 passes, all in the 4x perf mode (tensor_scalar / tensor_tensor
         mult only -- no bitwise xor, which runs 1x)
    PE:  diag cols bin (phase A) + [q;g] interleaved (phase B) + colsum
         + per-sample gbar stream
Sharding: pure data parallel, B=16 -> 2 samples on each of 8 cores; host
combines partial sums in float64.
"""

import sys

sys.path.insert(0, "/opt/trn_rl_repo")

from contextlib import ExitStack
from dataclasses import dataclass

import numpy as np

import concourse.bacc as bacc
import concourse.bass as bass
import concourse.mybir as mybir
import concourse.tile as tile

Act = mybir.ActivationFunctionType
Alu = mybir.AluOpType
BF16 = mybir.dt.bfloat16
FP16 = mybir.dt.float16
F32 = mybir.dt.float32

B, H, W = 16, 1024, 1024
NCORES = 8
SMOOTH = 1e-6
P = 128
DT = FP16  # on-device element dtype


@dataclass(frozen=True)
class Cfg:
    spc: int = B // NCORES   # samples per core
    sfree: int = 8192        # free columns per sample
    # per-sample tile widths: small first tile = fast pipeline ramp; the
    # second sample's plan is reversed so the drain tile is small too
    plan: tuple = (1024, 3072, 4096)

    @property
    def tps(self):  # tiles per sample
        return len(self.plan)

    @property
    def nt(self):  # tiles per core
        return self.tps * self.spc

    @property
    def px(self):  # pixels per sample
        return self.sfree * P

    @property
    def tiles(self):
        """[(sample, colstart, width), ...]"""
        assert sum(self.plan) == self.sfree
        out = []
        for s in range(self.spc):
            plan = self.plan if s == 0 else tuple(reversed(self.plan))
            c = 0
            for w in plan:
                out.append((s, c, w))
                c += w
        return out


CFG = Cfg()


def _table_id(nc, func) -> int:
    from concourse.hw_specs import get_activation_tables

    tables = get_activation_tables(nc.m.arch)
    for idx, (name, funcs) in enumerate(tables.items()):
        if func in funcs:
            return idx
    raise RuntimeError(f"no activation table with {func}")


def build_bass(cfg: Cfg = CFG, num_devices: int = NCORES) -> bass.Bass:
    nc = bacc.Bacc(
        "TRN2", target_bir_lowering=False, debug=False, num_devices=num_devices
    )
    x_d = nc.dram_tensor("x", [cfg.spc, P, cfg.sfree], F32, kind="ExternalInput").ap()
    t_d = nc.dram_tensor("t", [cfg.spc, P, cfg.sfree], F32, kind="ExternalInput").ap()
    # accum columns: [q, l] per tile (ACT) and [bin] per tile (DVE)
    aact_d = nc.dram_tensor("aact", [P, 2 * cfg.nt], F32, kind="ExternalOutput").ap()
    adve_d = nc.dram_tensor("adve", [P, cfg.nt], F32, kind="ExternalOutput").ap()
    # per-sample: diag blocks [sbar*bin, sbar*q, sbar*g], col sums of sbar,
    # and the gbar global-sum row
    diag_d = nc.dram_tensor("diag", [cfg.spc, P, 3, P], F32, kind="ExternalOutput").ap()
    ssum_d = nc.dram_tensor("ssum", [cfg.spc, P, 1], F32, kind="ExternalOutput").ap()
    gsum_d = nc.dram_tensor("gsum", [cfg.spc, 512], F32, kind="ExternalOutput").ap()

    with tile.TileContext(nc) as tc, ExitStack() as ctx:
        _emit(ctx, tc, cfg, x_d, t_d, aact_d, adve_d, diag_d, ssum_d, gsum_d)
    # The phase-separated emission (all Sigmoids, then all Lns) lets the
    # automatic table-load pass insert exactly two ATLs — no thrash.
    nc.compile()
    return nc


def _emit(ctx, tc, cfg: Cfg, x_d, t_d, aact_d, adve_d, diag_d, ssum_d, gsum_d):
    nc = tc.nc

    xpool = ctx.enter_context(tc.tile_pool(name="xp", bufs=2))   # x, then l
    tpool = ctx.enter_context(tc.tile_pool(name="tp", bufs=2))   # t, then rbar
    zpool = ctx.enter_context(tc.tile_pool(name="zp", bufs=2))   # zbar, then u
    bpool = ctx.enter_context(tc.tile_pool(name="bp", bufs=2))   # bin
    # sbar/qg persist from phase A to B. One pool per tile width: the
    # reversed second-sample plan means exactly 2 tiles per width, so
    # bufs=2 pools hold them with no recycling and no max-size waste.
    spool = {
        w: ctx.enter_context(tc.tile_pool(name=f"sp{w}", bufs=2))
        for w in set(cfg.plan)
    }
    qgpool = {
        w: ctx.enter_context(tc.tile_pool(name=f"qg{w}", bufs=2))
        for w in set(cfg.plan)
    }
    rpool = {
        w: ctx.enter_context(tc.tile_pool(name=f"rp{w}", bufs=2))
        for w in set(cfg.plan)
    }
    accpool = ctx.enter_context(tc.tile_pool(name="ac", bufs=1))
    stagepool = ctx.enter_context(tc.tile_pool(name="st", bufs=2))
    psumpool = ctx.enter_context(tc.tile_pool(name="ps", bufs=1, space="PSUM"))

    acc_act = accpool.tile([P, 2 * cfg.nt], F32)
    acc_dve = accpool.tile([P, cfg.nt], F32)
    ones = accpool.tile([P, 1], DT)
    nc.vector.memset(ones[:], 1.0)

    diag = [psumpool.tile([P, 3, P], F32, name=f"dg{s}") for s in range(cfg.spc)]
    ssum = [psumpool.tile([P, 1], F32, name=f"ss{s}") for s in range(cfg.spc)]
    gacc = [psumpool.tile([1, 512], F32, name=f"ga{s}") for s in range(cfg.spc)]

    sbar = [None] * cfg.nt
    qg = [None] * cfg.nt
    rbar = [None] * cfg.nt

    def _emit_rbar(k):
        _, _, kfw = cfg.tiles[k]
        rb = rpool[kfw].tile([P, kfw], DT, name=f"rb{k}", tag="rb")
        rbar[k] = rb
        nc.vector.tensor_scalar(
            out=rb[:], in0=qg[k][:, 0, :], scalar1=-1.0, scalar2=1.0,
            op0=Alu.mult, op1=Alu.add,
        )

    # ---------------- phase A: sigmoid table ----------------
    for i, (s, c0, fw) in enumerate(cfg.tiles):
        sl = slice(c0, c0 + fw)
        first = c0 == 0
        last = c0 + fw == cfg.sfree

        tb = tpool.tile([P, fw], DT, name=f"tb{i}", tag="tb")
        nc.gpsimd.dma_start(out=tb[:], in_=t_d[s][:, sl])  # casts fp32->fp16
        xb = xpool.tile([P, fw], DT, name=f"xb{i}", tag="xb")
        nc.gpsimd.dma_start(out=xb[:], in_=x_d[s][:, sl])

        # sbar = 2t-1 (exact in fp16); zbar = sbar*x (exact sign flip)
        sb = spool[fw].tile([P, fw], DT, name=f"sb{i}", tag="sb")
        nc.vector.tensor_scalar(
            out=sb[:], in0=tb[:], scalar1=2.0, scalar2=-1.0,
            op0=Alu.mult, op1=Alu.add,
        )
        zb = zpool.tile([P, fw], DT, name=f"zb{i}", tag="zb")
        nc.vector.tensor_tensor(out=zb[:], in0=sb[:], in1=xb[:], op=Alu.mult)
        bb = bpool.tile([P, fw], DT, name=f"bb{i}", tag="bb")
        nc.vector.tensor_scalar(
            out=bb[:], in0=xb[:], scalar1=0.0, scalar2=None,
            op0=Alu.is_gt, op1=Alu.add, accum_out=acc_dve[:, i : i + 1],
        )

        qgt = qgpool[fw].tile([P, 2, fw], DT, name=f"qg{i}", tag="qg")
        qg[i] = qgt
        sbar[i] = sb
        # q = sigmoid(zbar) = p_t
        nc.scalar.activation(
            out=qgt[:, 0, :], in_=zb[:], func=Act.Sigmoid,
            accum_out=acc_act[:, i : i + 1],
        )
        # rbar = 1-q in phase A (balances DVE across phases), emitted with
        # a one-tile skew so DVE's s/z/bin of tile i+1 fills the ACT latency
        if i > 0:
            _emit_rbar(i - 1)
        if i == cfg.nt - 1:
            _emit_rbar(i)

        # PE: diag block of sbar x bin, and column sums of sbar
        for j in range(fw // P):
            cs = slice(j * P, (j + 1) * P)
            nc.tensor.matmul(
                out=diag[s][:, 0, :], lhsT=sb[:, cs], rhs=bb[:, cs],
                start=(first and j == 0), stop=(last and j == fw // P - 1),
            )
            nc.tensor.matmul(
                out=ssum[s][:], lhsT=sb[:, cs], rhs=ones[:],
                start=(first and j == 0), stop=(last and j == fw // P - 1),
            )

    # ---------------- phase B: natural-log table ----------------
    for i, (s, c0, fw) in enumerate(cfg.tiles):
        first = c0 == 0
        last = c0 + fw == cfg.sfree
        qgt = qg[i]
        sb = sbar[i]

        # l = ln(q) = -ce
        lb = xpool.tile([P, fw], DT, name=f"lb{i}", tag="xb")
        nc.scalar.activation(
            out=lb[:], in_=qgt[:, 0, :], func=Act.Ln,
            accum_out=acc_act[:, cfg.nt + i : cfg.nt + i + 1],
        )
        # u = l*rbar ; gbar = u*rbar = -ce*r^2
        rb = rbar[i]
        ub = zpool.tile([P, fw], DT, name=f"ub{i}", tag="zb")
        nc.vector.tensor_tensor(out=ub[:], in0=lb[:], in1=rb[:], op=Alu.mult)
        nc.vector.tensor_tensor(out=qgt[:, 1, :], in0=ub[:], in1=rb[:], op=Alu.mult)

        # PE: diag blocks sbar x [q; gbar]
        for j in range(fw // P):
            cs = slice(j * P, (j + 1) * P)
            nc.tensor.matmul(
                out=diag[s][:, 1:3, :], lhsT=sb[:, cs], rhs=qgt[:, :, cs],
                start=(first and j == 0), stop=(last and j == fw // P - 1),
            )
        # PE: per-sample global sum of gbar (ones stationary, 512-col chunks)
        for j in range(fw // 512):
            nc.tensor.matmul(
                out=gacc[s][:], lhsT=ones[:],
                rhs=qgt[:, 1, j * 512 : (j + 1) * 512],
                start=(first and j == 0),
                stop=(last and j == fw // 512 - 1),
            )

    # ---------------- epilogue ----------------
    # Copies sit on the ACT queue AFTER all Lns: sample 0's run instantly,
    # only sample 1's wait on the final PE stop.
    for s in range(cfg.spc):
        dstage = stagepool.tile([P, 3, P], F32, name=f"dst{s}", tag="dst")
        nc.scalar.copy(out=dstage[:], in_=diag[s][:])
        nc.sync.dma_start(out=diag_d[s], in_=dstage[:])
        sstage = stagepool.tile([P, 1], F32, name=f"sst{s}", tag="sst")
        nc.scalar.copy(out=sstage[:], in_=ssum[s][:])
        nc.sync.dma_start(out=ssum_d[s], in_=sstage[:])
        gstage = stagepool.tile([1, 512], F32, name=f"gst{s}", tag="gst")
        nc.scalar.copy(out=gstage[:], in_=gacc[s][:])
        nc.sync.dma_start(out=gsum_d[s : s + 1], in_=gstage[:])
    nc.sync.dma_start(out=aact_d[:], in_=acc_act[:])
    nc.sync.dma_start(out=adve_d[:], in_=acc_dve[:])


def host_reduce(results, pred_iou, cfg: Cfg = CFG, ncores: int = NCORES):
    """Combine per-core partial sums into the final scalar loss (float64)."""
    n = float(cfg.px)                      # pixels per sample
    n_total = n * cfg.spc * ncores
    piou = np.asarray(pred_iou, np.float64).reshape(-1)

    ce_tot = 0.0
    g_tot = 0.0
    gt_tot = 0.0
    dice_terms = []
    iou_sq = []

    for c in range(ncores):
        aact = np.asarray(results[c]["aact"], np.float64).sum(axis=0)
        adve = np.asarray(results[c]["adve"], np.float64).sum(axis=0)
        diag = np.asarray(results[c]["diag"], np.float64)   # [spc, P, 3, P]
        ssum = np.asarray(results[c]["ssum"], np.float64)   # [spc, P]
        gsum = np.asarray(results[c]["gsum"], np.float64)   # [spc, 512]

        for s in range(cfg.spc):
            ti = [s * cfg.tps + k for k in range(cfg.tps)]
            q_s = sum(aact[i] for i in ti)                  # sum p_t
            l_s = sum(aact[cfg.nt + i] for i in ti)         # -sum ce
            bin_s = sum(adve[i] for i in ti)
            sb_s = float(ssum[s].sum())                     # sum sbar = 2*sum t - n
            t_s = 0.5 * (sb_s + n)
            sbin = np.trace(diag[s, :, 0, :])               # sum sbar*bin
            sq = np.trace(diag[s, :, 1, :])                 # sum sbar*q
            sg = np.trace(diag[s, :, 2, :])                 # sum sbar*gbar
            gbar_s = float(gsum[s].sum())                   # sum gbar = -sum g

            bint_s = 0.5 * (sbin + bin_s)                   # sum t*bin
            qt_s = 0.5 * (sq + q_s)                         # sum t*q
            g_s = -gbar_s                                   # sum g
            gt_s = -0.5 * (sg + gbar_s)                     # sum t*g
            ce_s = -l_s

            ce_tot += ce_s
            g_tot += g_s
            gt_tot += gt_s

            # dice: r = 1-q ; sum r = n - q_s ; sum rt = t_s - qt_s
            r_s = n - q_s
            rt_s = t_s - qt_s
            inter = t_s - rt_s                              # sum p*t
            p_sum = r_s + t_s - 2.0 * rt_s                  # sum p
            union = p_sum + t_s
            dice_terms.append((2.0 * inter + SMOOTH) / (union + SMOOTH))

            uni = bin_s + t_s - bint_s
            aiou = (bint_s + SMOOTH) / (uni + SMOOTH)
            gidx = c * cfg.spc + s
            iou_sq.append((piou[gidx] - aiou) ** 2)

    focal = (0.75 * g_tot - 0.5 * gt_tot) / n_total
    dice = 1.0 - float(np.mean(dice_terms))
    boundary = 2.0 * ce_tot / n_total
    iou_loss = float(np.mean(iou_sq))
    total = focal + dice + 0.5 * boundary + 0.1 * iou_loss
    return np.array(total, dtype=np.float32)


_NC_CACHE = {}


def _get_nc(cfg: Cfg = CFG):
    key = (cfg.spc, cfg.plan)
    if key not in _NC_CACHE:
        _NC_CACHE[key] = build_bass(cfg)
    return _NC_CACHE[key]


def make_in_maps(pred_masks, gt_masks, cfg: Cfg = CFG, ncores: int = NCORES):
    x = np.ascontiguousarray(pred_masks, dtype=np.float32).reshape(
        ncores, cfg.spc, P, cfg.sfree
    )
    t = np.ascontiguousarray(gt_masks, dtype=np.float32).reshape(
        ncores, cfg.spc, P, cfg.sfree
    )
    return [{"x": x[c], "t": t[c]} for c in range(ncores)]


def kernel(pred_masks, gt_masks, pred_iou):
    from concourse.bass_utils import run_bass_kernel_spmd

    nc = _get_nc()
    in_maps = make_in_maps(pred_masks, gt_masks)
    res = run_bass_kernel_spmd(nc, in_maps, core_ids=list(range(NCORES)))
    return host_reduce(res.results, pred_iou)


# revision 40
# speedup vs baseline: 1.1380x; 1.1380x over previous
"""Trainium2 Bass kernel for CellSegmentationLoss.

For pred_masks x (logits, fp32 [16,1,1024,1024]), gt_masks t (binary fp32),
pred_iou [16,1]:

    ce    = softplus(x) - x*t
    p     = sigmoid(x)
    focal = mean(alpha_t * ce * (1-p_t)^2),  alpha_t = 0.75-0.5t
    dice  = 1 - mean_s (2*sum(p*t)+eps)/(sum(p)+sum(t)+eps)
    boundary = 2*mean(ce)
    iou_loss = mse(pred_iou, actual_iou of thresholded masks)
    loss  = focal + dice + 0.5*boundary + 0.1*iou_loss

Identity (t binary): with zbar = (2t-1)*x
    q  = sigmoid(zbar) = p_t          (ACT pass 1, accum -> sum q)
    l  = ln(q)        = -ce           (ACT pass 2, accum -> -sum ce)
    r  = 1 - q        = sigmoid(-zbar)
    u  = l*r = -ce*r ; gbar = u*r = -ce*r^2 = -(focal modulator g)
    bin = (x > 0) = (p > 0.5)

All needed reductions:
    global: sum q, sum l, per-sample: sum bin (DVE/ACT accum_out, free)
    per-sample sum gbar (PE, ones-stationary streaming gbar)
    t-weighted per-sample sums via PE diag matmuls with STATIONARY
    sbar = 2t-1 (exact +-1): sum sbar*v = 2*sum(t*v) - sum(v), plus a
    1-column ones matmul per chunk giving column sums of sbar -> sum t.

Engine plan per core (2 samples, 16384 free columns):
    ACT: 2 passes (Sigmoid table, then Ln table; one explicit table load
         each -> no thrash)
    DVE: 6 passes, all in the 4x perf mode (tensor_scalar / tensor_tensor
         mult only -- no bitwise xor, which runs 1x)
    PE:  diag cols bin (phase A) + [q;g] interleaved (phase B) + colsum
         + per-sample gbar stream
Sharding: pure data parallel, B=16 -> 2 samples on each of 8 cores; host
combines partial sums in float64.
"""

import sys

sys.path.insert(0, "/opt/trn_rl_repo")

from contextlib import ExitStack
from dataclasses import dataclass

import numpy as np

import concourse.bacc as bacc
import concourse.bass as bass
import concourse.mybir as mybir
import concourse.tile as tile

Act = mybir.ActivationFunctionType
Alu = mybir.AluOpType
BF16 = mybir.dt.bfloat16
FP16 = mybir.dt.float16
F32 = mybir.dt.float32

B, H, W = 16, 1024, 1024
NCORES = 8
SMOOTH = 1e-6
P = 128
DT = FP16  # on-device element dtype


@dataclass(frozen=True)
class Cfg:
    spc: int = B // NCORES   # samples per core
    sfree: int = 8192        # free columns per sample
    # per-sample tile widths: small first tile = fast pipeline ramp; the
    # second sample's plan is reversed so the drain tile is small too
    plan: tuple = (512, 1536, 2560, 3584)

    @property
    def tps(self):  # tiles per sample
        return len(self.plan)

    @property
    def nt(self):  # tiles per core
        return self.tps * self.spc

    @property
    def px(self):  # pixels per sample
        return self.sfree * P

    @property
    def tiles(self):
        """[(sample, colstart, width), ...]"""
        assert sum(self.plan) == self.sfree
        out = []
        for s in range(self.spc):
            plan = self.plan if s == 0 else tuple(reversed(self.plan))
            c = 0
            for w in plan:
                out.append((s, c, w))
                c += w
        return out


CFG = Cfg()


def _table_id(nc, func) -> int:
    from concourse.hw_specs import get_activation_tables

    tables = get_activation_tables(nc.m.arch)
    for idx, (name, funcs) in enumerate(tables.items()):
        if func in funcs:
            return idx
    raise RuntimeError(f"no activation table with {func}")


def build_bass(cfg: Cfg = CFG, num_devices: int = NCORES) -> bass.Bass:
    nc = bacc.Bacc(
        "TRN2", target_bir_lowering=False, debug=False, num_devices=num_devices
    )
    x_d = nc.dram_tensor("x", [cfg.spc, P, cfg.sfree], F32, kind="ExternalInput").ap()
    t_d = nc.dram_tensor("t", [cfg.spc, P, cfg.sfree], F32, kind="ExternalInput").ap()
    # accum columns: [q, l] per tile (ACT) and [bin, sbar] per tile (DVE)
    aact_d = nc.dram_tensor("aact", [P, 2 * cfg.nt], F32, kind="ExternalOutput").ap()
    adve_d = nc.dram_tensor("adve", [P, cfg.nt], F32, kind="ExternalOutput").ap()
    ssum_d = nc.dram_tensor("ssum", [cfg.spc, P, 1], F32, kind="ExternalOutput").ap()
    # per-sample: diag blocks [sbar*bin, sbar*q, sbar*g] and the gbar row
    diag_d = nc.dram_tensor("diag", [cfg.spc, P, 3, P], F32, kind="ExternalOutput").ap()
    gsum_d = nc.dram_tensor("gsum", [cfg.spc, 512], F32, kind="ExternalOutput").ap()

    with tile.TileContext(nc) as tc, ExitStack() as ctx:
        _emit(ctx, tc, cfg, x_d, t_d, aact_d, adve_d, diag_d, ssum_d, gsum_d)
    # The phase-separated emission (all Sigmoids, then all Lns) lets the
    # automatic table-load pass insert exactly two ATLs -- no thrash.
    nc.compile()
    return nc


def _emit(ctx, tc, cfg: Cfg, x_d, t_d, aact_d, adve_d, diag_d, ssum_d, gsum_d):
    nc = tc.nc

    xpool = ctx.enter_context(tc.tile_pool(name="xp", bufs=2))   # x, then l
    tpool = ctx.enter_context(tc.tile_pool(name="tp", bufs=2))   # t
    zpool = ctx.enter_context(tc.tile_pool(name="zp", bufs=2))   # zbar, then u
    gpool = ctx.enter_context(tc.tile_pool(name="gp", bufs=2))   # gbar
    # sbar, [bin; q], and rbar persist from phase A to B. One pool per
    # tile width: the reversed second-sample plan means exactly 2 tiles
    # per width, so bufs=2 pools hold them with no recycling.
    spool = {
        w: ctx.enter_context(tc.tile_pool(name=f"sp{w}", bufs=2))
        for w in set(cfg.plan)
    }
    bqpool = {
        w: ctx.enter_context(tc.tile_pool(name=f"bq{w}", bufs=2))
        for w in set(cfg.plan)
    }
    rpool = {
        w: ctx.enter_context(tc.tile_pool(name=f"rp{w}", bufs=2))
        for w in set(cfg.plan)
    }
    accpool = ctx.enter_context(tc.tile_pool(name="ac", bufs=1))
    stagepool = ctx.enter_context(tc.tile_pool(name="st", bufs=2))
    psumpool = [
        ctx.enter_context(tc.tile_pool(name=f"ps{s}", bufs=1, space="PSUM"))
        for s in range(cfg.spc)
    ]

    acc_act = accpool.tile([P, 2 * cfg.nt], F32)
    acc_dve = accpool.tile([P, cfg.nt], F32)
    ones = accpool.tile([P, 1], DT)
    nc.vector.memset(ones[:], 1.0)

    diag = [psumpool[s].tile([P, 3, P], F32, name=f"dg{s}") for s in range(cfg.spc)]
    ssum = [psumpool[s].tile([P, 1], F32, name=f"ss{s}") for s in range(cfg.spc)]
    gacc = [psumpool[s].tile([1, 512], F32, name=f"ga{s}") for s in range(cfg.spc)]

    sbar = [None] * cfg.nt
    bq = [None] * cfg.nt
    rbar = [None] * cfg.nt

    def _emit_rbar(k):
        _, _, kfw = cfg.tiles[k]
        rb = rpool[kfw].tile([P, kfw], DT, name=f"rb{k}", tag="rb")
        rbar[k] = rb
        nc.vector.tensor_scalar(
            out=rb[:], in0=bq[k][:, 1, :], scalar1=-1.0, scalar2=1.0,
            op0=Alu.mult, op1=Alu.add,
        )

    # ---------------- phase A: sigmoid table ----------------
    for i, (s, c0, fw) in enumerate(cfg.tiles):
        sl = slice(c0, c0 + fw)
        first = c0 == 0
        last = c0 + fw == cfg.sfree

        tb = tpool.tile([P, fw], DT, name=f"tb{i}", tag="tb")
        nc.gpsimd.dma_start(out=tb[:], in_=t_d[s][:, sl])  # casts fp32->fp16
        xb = xpool.tile([P, fw], DT, name=f"xb{i}", tag="xb")
        nc.gpsimd.dma_start(out=xb[:], in_=x_d[s][:, sl])

        # sbar = 2t-1 (exact in fp16); accum gives sum(sbar) -> sum(t)
        sb = spool[fw].tile([P, fw], DT, name=f"sb{i}", tag="sb")
        nc.vector.tensor_scalar(
            out=sb[:], in0=tb[:], scalar1=2.0, scalar2=-1.0,
            op0=Alu.mult, op1=Alu.add,
        )
        sbar[i] = sb
        bqt = bqpool[fw].tile([P, 2, fw], DT, name=f"bq{i}", tag="bq")
        bq[i] = bqt
        # zbar = sbar*x (exact sign flip); bin = (x>0)
        zb = zpool.tile([P, fw], DT, name=f"zb{i}", tag="zb")
        nc.vector.tensor_tensor(out=zb[:], in0=sb[:], in1=xb[:], op=Alu.mult)
        nc.vector.tensor_scalar(
            out=bqt[:, 0, :], in0=xb[:], scalar1=0.0, scalar2=None,
            op0=Alu.is_gt, op1=Alu.add, accum_out=acc_dve[:, i : i + 1],
        )
        # q = sigmoid(zbar) = p_t
        nc.scalar.activation(
            out=bqt[:, 1, :], in_=zb[:], func=Act.Sigmoid,
            accum_out=acc_act[:, i : i + 1],
        )
        # rbar = 1-q in phase A (balances DVE across phases), with a
        # one-tile skew so DVE s/z/bin of tile i+1 fills the ACT latency
        if i > 0:
            _emit_rbar(i - 1)
        if i == cfg.nt - 1:
            _emit_rbar(i)

        # PE: diag blocks sbar x [bin; q] -- overlapped with phase A
        for j in range(fw // P):
            cs = slice(j * P, (j + 1) * P)
            nc.tensor.matmul(
                out=diag[s][:, 0:2, :], lhsT=sb[:, cs], rhs=bqt[:, :, cs],
                start=(first and j == 0), stop=(last and j == fw // P - 1),
            )
            nc.tensor.matmul(
                out=ssum[s][:], lhsT=sb[:, cs], rhs=ones[:],
                start=(first and j == 0), stop=(last and j == fw // P - 1),
            )

    # ---------------- phase B: natural-log table ----------------
    for i, (s, c0, fw) in enumerate(cfg.tiles):
        first = c0 == 0
        last = c0 + fw == cfg.sfree
        sb = sbar[i]

        # l = ln(q) = -ce
        lb = xpool.tile([P, fw], DT, name=f"lb{i}", tag="xb")
        nc.scalar.activation(
            out=lb[:], in_=bq[i][:, 1, :], func=Act.Ln,
            accum_out=acc_act[:, cfg.nt + i : cfg.nt + i + 1],
        )
        # u = l*rbar ; gbar = u*rbar = -ce*r^2. Column-split 3:1 between
        # DVE (2x mode, 0.54 ns/el) and the otherwise-idle Pool engine
        # (Q7 multiply, ~2 ns/el) to shave the DVE-bound phase-B span.
        rb = rbar[i]
        fs = (fw * 13 // 16) // P * P
        ub = zpool.tile([P, fw], DT, name=f"ub{i}", tag="zb")
        nc.vector.tensor_tensor(
            out=ub[:, :fs], in0=lb[:, :fs], in1=rb[:, :fs], op=Alu.mult)
        nc.gpsimd.tensor_tensor(
            out=ub[:, fs:], in0=lb[:, fs:], in1=rb[:, fs:], op=Alu.mult)
        gb = gpool.tile([P, fw], DT, name=f"gb{i}", tag="gb")
        nc.vector.tensor_tensor(
            out=gb[:, :fs], in0=ub[:, :fs], in1=rb[:, :fs], op=Alu.mult)
        nc.gpsimd.tensor_tensor(
            out=gb[:, fs:], in0=ub[:, fs:], in1=rb[:, fs:], op=Alu.mult)

        # PE: diag block sbar x gbar, and the per-sample gbar global sum
        for j in range(fw // P):
            cs = slice(j * P, (j + 1) * P)
            nc.tensor.matmul(
                out=diag[s][:, 2, :], lhsT=sb[:, cs], rhs=gb[:, cs],
                start=(first and j == 0), stop=(last and j == fw // P - 1),
            )
        for j in range(fw // 512):
            nc.tensor.matmul(
                out=gacc[s][:], lhsT=ones[:],
                rhs=gb[:, j * 512 : (j + 1) * 512],
                start=(first and j == 0),
                stop=(last and j == fw // 512 - 1),
            )

    # ---------------- epilogue ----------------
    # Copies sit on the ACT queue AFTER all Lns: sample 0's run instantly,
    # only sample 1's wait on the final PE stop.
    for s in range(cfg.spc):
        dstage = stagepool.tile([P, 3, P], F32, name=f"dst{s}", tag="dst")
        nc.scalar.copy(out=dstage[:], in_=diag[s][:])
        nc.sync.dma_start(out=diag_d[s], in_=dstage[:])
        sstage = stagepool.tile([P, 1], F32, name=f"sst{s}", tag="sst")
        nc.scalar.copy(out=sstage[:], in_=ssum[s][:])
        nc.sync.dma_start(out=ssum_d[s], in_=sstage[:])
        gstage = stagepool.tile([1, 512], F32, name=f"gst{s}", tag="gst")
        nc.scalar.copy(out=gstage[:], in_=gacc[s][:])
        nc.sync.dma_start(out=gsum_d[s : s + 1], in_=gstage[:])
    nc.sync.dma_start(out=aact_d[:], in_=acc_act[:])
    nc.sync.dma_start(out=adve_d[:], in_=acc_dve[:])


def host_reduce(results, pred_iou, cfg: Cfg = CFG, ncores: int = NCORES):
    """Combine per-core partial sums into the final scalar loss (float64)."""
    n = float(cfg.px)                      # pixels per sample
    n_total = n * cfg.spc * ncores
    piou = np.asarray(pred_iou, np.float64).reshape(-1)

    ce_tot = 0.0
    g_tot = 0.0
    gt_tot = 0.0
    dice_terms = []
    iou_sq = []

    for c in range(ncores):
        aact = np.asarray(results[c]["aact"], np.float64).sum(axis=0)
        adve = np.asarray(results[c]["adve"], np.float64).sum(axis=0)
        diag = np.asarray(results[c]["diag"], np.float64)   # [spc, P, 3, P]
        ssum = np.asarray(results[c]["ssum"], np.float64)   # [spc, P, 1]
        gsum = np.asarray(results[c]["gsum"], np.float64)   # [spc, 512]

        for s in range(cfg.spc):
            ti = [s * cfg.tps + k for k in range(cfg.tps)]
            q_s = sum(aact[i] for i in ti)                  # sum p_t
            l_s = sum(aact[cfg.nt + i] for i in ti)         # -sum ce
            bin_s = sum(adve[i] for i in ti)
            sb_s = float(ssum[s].sum())                     # sum sbar = 2*sum t - n
            t_s = 0.5 * (sb_s + n)
            sbin = np.trace(diag[s, :, 0, :])               # sum sbar*bin
            sq = np.trace(diag[s, :, 1, :])                 # sum sbar*q
            sg = np.trace(diag[s, :, 2, :])                 # sum sbar*gbar
            gbar_s = float(gsum[s].sum())                   # sum gbar = -sum g

            bint_s = 0.5 * (sbin + bin_s)                   # sum t*bin
            qt_s = 0.5 * (sq + q_s)                         # sum t*q
            g_s = -gbar_s                                   # sum g
            gt_s = -0.5 * (sg + gbar_s)                     # sum t*g
            ce_s = -l_s

            ce_tot += ce_s
            g_tot += g_s
            gt_tot += gt_s

            # dice: r = 1-q ; sum r = n - q_s ; sum rt = t_s - qt_s
            r_s = n - q_s
            rt_s = t_s - qt_s
            inter = t_s - rt_s                              # sum p*t
            p_sum = r_s + t_s - 2.0 * rt_s                  # sum p
            union = p_sum + t_s
            dice_terms.append((2.0 * inter + SMOOTH) / (union + SMOOTH))

            uni = bin_s + t_s - bint_s
            aiou = (bint_s + SMOOTH) / (uni + SMOOTH)
            gidx = c * cfg.spc + s
            iou_sq.append((piou[gidx] - aiou) ** 2)

    focal = (0.75 * g_tot - 0.5 * gt_tot) / n_total
    dice = 1.0 - float(np.mean(dice_terms))
    boundary = 2.0 * ce_tot / n_total
    iou_loss = float(np.mean(iou_sq))
    total = focal + dice + 0.5 * boundary + 0.1 * iou_loss
    return np.array(total, dtype=np.float32)


_NC_CACHE = {}


def _get_nc(cfg: Cfg = CFG):
    key = (cfg.spc, cfg.plan)
    if key not in _NC_CACHE:
        _NC_CACHE[key] = build_bass(cfg)
    return _NC_CACHE[key]


def make_in_maps(pred_masks, gt_masks, cfg: Cfg = CFG, ncores: int = NCORES):
    x = np.ascontiguousarray(pred_masks, dtype=np.float32).reshape(
        ncores, cfg.spc, P, cfg.sfree
    )
    t = np.ascontiguousarray(gt_masks, dtype=np.float32).reshape(
        ncores, cfg.spc, P, cfg.sfree
    )
    return [{"x": x[c], "t": t[c]} for c in range(ncores)]


def kernel(pred_masks, gt_masks, pred_iou):
    from concourse.bass_utils import run_bass_kernel_spmd

    nc = _get_nc()
    in_maps = make_in_maps(pred_masks, gt_masks)
    res = run_bass_kernel_spmd(nc, in_maps, core_ids=list(range(NCORES)))
    return host_reduce(res.results, pred_iou)
